# revision 18
# baseline (speedup 1.0000x reference)
"""Trainium2 Bass kernel for NoSharingGraphConv.

out[b,w,m] = sum_{h,n} x[b,h,n] * adj[h,w] * W[h,w,n,m] + bias[m]
  B=4096, N=17 (graph nodes), FIN=FOUT=256.

Sharding (8 NeuronCores): 4 batch groups x 2 out-feature halves.
Core c handles batch rows [bg*1024, (bg+1)*1024) and out features
[mh*128, (mh+1)*128), bg = c>>1, mh = c&1. This halves the per-core W
stream (37.9MB) vs pure batch-parallel while keeping the PE work
perfectly balanced (1156 matmuls of [128x128]x[128x512] per core).

Device kernel (per core):
  - x^T shard resident in SBUF [128, 34, 1024] (host-transposed, n
    interleaved as n = 2p+kc so it matches the W slab layout).
  - W streamed one w-slab at a time; host pre-swizzles W into the exact
    slab layout [w, p, h, kc, m'] so each slab DMA is one fully
    contiguous 2.2MB read (17.4KB per partition line).
  - Slab scaled in-place by adj[:,w] on the DVE (per-h tensor_scalar,
    2x mode); adj is broadcast across partitions once via gpsimd.
  - Per (w, batch-half): 34 accumulating float32r matmuls into one PSUM
    bank; ACT evacuates with the per-partition bias add (fp32).
  - Device writes out_t [17, 128, 1024] (w, m', b); host permutes back.
"""

import sys

if "/opt/trn_rl_repo" not in sys.path:
    sys.path.insert(0, "/opt/trn_rl_repo")

import numpy as np

B, N, FIN, FOUT = 4096, 17, 256, 256
NC = 8
NBG = 4  # batch groups
BS = B // NBG  # 1024 batch rows per core
MH = FOUT // 2  # 128 out features per core
KCH = N * FIN // 128  # 34 contraction chunks of 128
NBH = BS // 512  # 2 batch halves (matmul free dim 512)

_CACHE = {}


def _build_module():
    import concourse.mybir as mybir
    import concourse.tile as tile
    from concourse import bacc

    f32 = mybir.dt.float32
    f32r = mybir.dt.float32r

    nc = bacc.Bacc("TRN2", target_bir_lowering=False)

    xt_d = nc.dram_tensor("xt", [N * FIN, BS], f32r, kind="ExternalInput")
    # host-swizzled: w_sw[w, p, h, kc, m'] = adj-unscaled W[h, w, 2p+kc, mh*128+m']
    w_d = nc.dram_tensor("w_sw", [N, 128, N, 2, MH], f32r, kind="ExternalInput")
    adj_d = nc.dram_tensor("adj", [N, N], f32, kind="ExternalInput")
    b_d = nc.dram_tensor("b", [MH], f32, kind="ExternalInput")
    o_d = nc.dram_tensor("out_t", [N, MH, BS], f32, kind="ExternalOutput")

    with tile.TileContext(nc) as tc:
        with (
            tc.tile_pool(name="const", bufs=1) as const,
            tc.tile_pool(name="wslab", bufs=2) as wpool,
            tc.tile_pool(name="obuf", bufs=4) as opool,
            tc.tile_pool(name="psum", bufs=6, space="PSUM") as psum,
        ):
            # adj, transposed to (w, h) order, on partition 0, then
            # replicated across all 128 partitions (gpsimd custom inst)
            adj_row = const.tile([1, N, N], f32)
            nc.sync.dma_start(adj_row[:], adj_d[:].rearrange("h w -> w h")[None])
            adj_sb = const.tile([128, N, N], f32)  # [p][w][h]
            nc.gpsimd.partition_broadcast(adj_sb[:], adj_row[:])

            # bias half on partitions: bias_sb[p, 0] = b[mh*128 + p]
            bias_sb = const.tile([128, 1], f32)
            nc.sync.dma_start(bias_sb[:], b_d[:][:, None])

            # resident x^T, host-permuted: chunk c=(h,kc) row p holds
            # x[b, h, 2p+kc]. First batch-half loaded up front; the
            # second half is emitted after the first two w slabs so the
            # first matmul groups aren't starved. (ACT ring, so w-slab
            # loads on the SP ring run in parallel.)
            xt_sb = const.tile([128, KCH, BS], f32r)
            xt_src = xt_d[:].rearrange("(c p) b -> p c b", p=128)
            for c0, c1 in ((0, 9), (9, 18), (18, 26), (26, KCH)):
                nc.scalar.dma_start(xt_sb[:, c0:c1, 0:512], xt_src[:, c0:c1, 0:512])

            def load_slab(w):
                # one fully-contiguous 2.2MB slab read, then per-h
                # adj-scale on the DVE (tensor_scalar = 2x mode)
                wt = wpool.tile([128, N, 2, MH], f32r, tag="wslab")
                nc.sync.dma_start(
                    wt[:].rearrange("p h kc m -> p (h kc m)"),
                    w_d[w].rearrange("p h kc m -> p (h kc m)"),
                )
                for h in range(N):
                    nc.vector.tensor_scalar_mul(
                        wt[:, h].rearrange("p kc m -> p (kc m)"),
                        wt[:, h].rearrange("p kc m -> p (kc m)"),
                        adj_sb[:, w, h : h + 1],
                    )
                return wt

            def mm_group(wt, w, bh):
                ps = psum.tile([128, 512], mybir.dt.float32, tag="ps")
                for c in range(KCH):
                    h, kc = divmod(c, 2)
                    nc.tensor.matmul(
                        ps[:],
                        lhsT=wt[:, h, kc, :],
                        rhs=xt_sb[:, c, bh * 512 : (bh + 1) * 512],
                        start=(c == 0),
                        stop=(c == KCH - 1),
                    )
                evac(ps, w, bh)

            def evac(ps, w, bh):
                ot = opool.tile([128, 512], f32, tag="ot")
                nc.scalar.activation(
                    ot[:],
                    ps[:],
                    mybir.ActivationFunctionType.Identity,
                    bias=bias_sb[:, 0:1],
                )
                nc.scalar.dma_start(o_d[w, :, bh * 512 : (bh + 1) * 512], ot[:])

            # w = 0, 1: batch-halves kept separate so the first groups
            # only need the first half of x^T (prologue is HBM-bound)
            wt0 = load_slab(0)
            wt1 = load_slab(1)
            for c0, c1 in ((0, 9), (9, 18), (18, 26), (26, KCH)):
                nc.scalar.dma_start(
                    xt_sb[:, c0:c1, 512:BS], xt_src[:, c0:c1, 512:BS]
                )
            mm_group(wt0, 0, 0)
            mm_group(wt1, 1, 0)
            mm_group(wt0, 0, 1)
            mm_group(wt1, 1, 1)

            # w >= 2: both batch-halves interleaved per contraction
            # chunk, so each weight load serves two matmuls (the 187ns
            # LDWEIGHTS hides under 2x213ns of streaming)
            for w in range(2, N):
                wt = load_slab(w)
                ps0 = psum.tile([128, 512], mybir.dt.float32, tag="ps")
                ps1 = psum.tile([128, 512], mybir.dt.float32, tag="ps")
                for c in range(KCH):
                    h, kc = divmod(c, 2)
                    for bh, ps in ((0, ps0), (1, ps1)):
                        nc.tensor.matmul(
                            ps[:],
                            lhsT=wt[:, h, kc, :],
                            rhs=xt_sb[:, c, bh * 512 : (bh + 1) * 512],
                            start=(c == 0),
                            stop=(c == KCH - 1),
                        )
                evac(ps0, w, 0)
                evac(ps1, w, 1)

    nc.compile()
    return nc


def _get_module():
    if "nc" not in _CACHE:
        _CACHE["nc"] = _build_module()
    return _CACHE["nc"]


def kernel(x, adj, W, b, _trace=False):
    from concourse.bass_utils import run_bass_kernel_spmd

    x = np.ascontiguousarray(np.asarray(x, dtype=np.float32))
    adj = np.ascontiguousarray(np.asarray(adj, dtype=np.float32))
    W = np.ascontiguousarray(np.asarray(W, dtype=np.float32))
    b = np.ascontiguousarray(np.asarray(b, dtype=np.float32))

    nc = _get_module()

    # W pre-swizzled per m-half: [w, p, h, kc, m'] = W[h, w, 2p+kc, mh*128+m']
    w_sw = []
    for mh in range(2):
        wh = W[:, :, :, mh * MH : (mh + 1) * MH]  # [h, w, n, m']
        wr = wh.reshape(N, N, FIN // 2, 2, MH)  # (h, w, p, kc, m')
        w_sw.append(np.ascontiguousarray(wr.transpose(1, 2, 0, 3, 4)))

    xt_by_bg = []
    for bg in range(NBG):
        xs = x[bg * BS : (bg + 1) * BS]  # [BS, N, FIN]
        # xt[(h*2+kc)*128 + p, b] = x[b, h, 2p+kc]
        xr = xs.reshape(BS, N, FIN // 2, 2)
        xt_by_bg.append(
            np.ascontiguousarray(xr.transpose(1, 3, 2, 0).reshape(N * FIN, BS))
        )

    in_maps = []
    for c in range(NC):
        bg, mh = divmod(c, 2)
        in_maps.append(
            {
                "xt": xt_by_bg[bg],
                "w_sw": w_sw[mh],
                "adj": adj,
                "b": b[mh * MH : (mh + 1) * MH].copy(),
            }
        )

    res = run_bass_kernel_spmd(nc, in_maps, list(range(NC)), trace=_trace)
    _CACHE["last_result"] = res

    out = np.empty((B, N, FOUT), dtype=np.float32)
    for c in range(NC):
        bg, mh = divmod(c, 2)
        ot = res.results[c]["out_t"]  # [17, 128, 1024] = (w, m', b)
        out[bg * BS : (bg + 1) * BS, :, mh * MH : (mh + 1) * MH] = ot.transpose(
            2, 0, 1
        )
    return out


# revision 26
# speedup vs baseline: 1.3185x; 1.3185x over previous
"""Trainium2 Bass kernel for NoSharingGraphConv.

out[b,w,m] = sum_{h,n} x[b,h,n] * adj[h,w] * W[h,w,n,m] + bias[m]
  B=4096, N=17 (graph nodes), FIN=FOUT=256.

Sharding (8 NeuronCores): 4 batch groups x 2 out-feature halves.
Core c handles batch rows [bg*1024, (bg+1)*1024) and out features
[mh*128, (mh+1)*128), bg = c>>1, mh = c&1. This halves the per-core W
stream (37.9MB) vs pure batch-parallel while keeping the PE work
perfectly balanced (1156 matmuls of [128x128]x[128x512] per core).

Device kernel (per core):
  - x^T shard resident in SBUF [128, 34, 1024] (host-transposed, n
    interleaved as n = 2p+kc so it matches the W slab layout).
  - W streamed one w-slab at a time; host pre-swizzles W into the exact
    slab layout [w, p, h, kc, m'] so each slab DMA is one fully
    contiguous 2.2MB read (17.4KB per partition line).
  - Slab scaled in-place by adj[:,w] on the DVE (per-h tensor_scalar,
    2x mode); adj is broadcast across partitions once via gpsimd.
  - Per (w, batch-half): 34 accumulating float32r matmuls into one PSUM
    bank; ACT evacuates with the per-partition bias add (fp32).
  - Device writes out_t [17, 128, 1024] (w, m', b); host permutes back.
"""

import sys

if "/opt/trn_rl_repo" not in sys.path:
    sys.path.insert(0, "/opt/trn_rl_repo")

import numpy as np

B, N, FIN, FOUT = 4096, 17, 256, 256
NC = 8
NBG = 4  # batch groups
BS = B // NBG  # 1024 batch rows per core
MH = FOUT // 2  # 128 out features per core
KCH = N * FIN // 128  # 34 contraction chunks of 128
NBH = BS // 512  # 2 batch halves (matmul free dim 512)

_CACHE = {}


def _build_module():
    import concourse.mybir as mybir
    import concourse.tile as tile
    from concourse import bacc

    f32 = mybir.dt.float32
    f32r = mybir.dt.float32r
    bf16 = mybir.dt.bfloat16

    nc = bacc.Bacc("TRN2", target_bir_lowering=False)

    # bf16 inputs: halves the dominant W DMA stream, halves the x^T
    # prologue load, and enables the PE fast-weight-load path (fp32
    # weight loads serialize at ~187ns/matmul). Accumulation stays fp32
    # in PSUM; walrus forbids mixing 16/32-bit matmul operands.
    xt_d = nc.dram_tensor("xt", [N * FIN, BS], bf16, kind="ExternalInput")
    # host-swizzled: w_sw[w, p, h, kc, m'] = bf16(W[h, w, 2p+kc, mh*128+m'])
    w_d = nc.dram_tensor("w_sw", [N, 128, N, 2, MH], bf16, kind="ExternalInput")
    adj_d = nc.dram_tensor("adj", [N, N], f32, kind="ExternalInput")
    b_d = nc.dram_tensor("b", [MH], f32, kind="ExternalInput")
    o_d = nc.dram_tensor("out_t", [N, MH, BS], f32, kind="ExternalOutput")

    with tile.TileContext(nc) as tc:
        with (
            tc.tile_pool(name="const", bufs=1) as const,
            tc.tile_pool(name="wslab", bufs=3) as wpool,
            tc.tile_pool(name="obuf", bufs=4) as opool,
            tc.tile_pool(name="psum", bufs=6, space="PSUM") as psum,
        ):
            # adj, transposed to (w, h) order, on partition 0, then
            # replicated across all 128 partitions (gpsimd custom inst)
            adj_row = const.tile([1, N, N], f32)
            nc.sync.dma_start(adj_row[:], adj_d[:].rearrange("h w -> w h")[None])
            adj_sb = const.tile([128, N, N], f32)  # [p][w][h]
            nc.gpsimd.partition_broadcast(adj_sb[:], adj_row[:])

            # bias half on partitions: bias_sb[p, 0] = b[mh*128 + p]
            bias_sb = const.tile([128, 1], f32)
            nc.sync.dma_start(bias_sb[:], b_d[:][:, None])

            # resident x^T, host-permuted: chunk c=(h,kc) row p holds
            # x[b, h, 2p+kc]. First batch-half loaded up front; the
            # second half is emitted after the first two w slabs so the
            # first matmul groups aren't starved. (ACT ring, so w-slab
            # loads on the SP ring run in parallel.)
            xt_sb = const.tile([128, KCH, BS], bf16)
            xt_src = xt_d[:].rearrange("(c p) b -> p c b", p=128)
            for c0, c1 in ((0, 9), (9, 18), (18, 26), (26, KCH)):
                nc.scalar.dma_start(xt_sb[:, c0:c1, 0:512], xt_src[:, c0:c1, 0:512])

            def load_slab(w):
                # one fully-contiguous 1.1MB slab read, then per-h
                # adj-scale on the DVE (bf16 tensor_scalar = 4x mode)
                wt = wpool.tile([128, N, 2, MH], bf16, tag="wslab")
                nc.sync.dma_start(
                    wt[:].rearrange("p h kc m -> p (h kc m)"),
                    w_d[w].rearrange("p h kc m -> p (h kc m)"),
                )
                for h in range(N):
                    nc.vector.tensor_scalar_mul(
                        wt[:, h].rearrange("p kc m -> p (kc m)"),
                        wt[:, h].rearrange("p kc m -> p (kc m)"),
                        adj_sb[:, w, h : h + 1],
                    )
                return wt

            def mm_group(wt, w, bh):
                ps = psum.tile([128, 512], mybir.dt.float32, tag="ps")
                for c in range(KCH):
                    h, kc = divmod(c, 2)
                    nc.tensor.matmul(
                        ps[:],
                        lhsT=wt[:, h, kc, :],
                        rhs=xt_sb[:, c, bh * 512 : (bh + 1) * 512],
                        start=(c == 0),
                        stop=(c == KCH - 1),
                    )
                evac(ps, w, bh)

            def evac(ps, w, bh):
                ot = opool.tile([128, 512], f32, tag="ot")
                nc.scalar.activation(
                    ot[:],
                    ps[:],
                    mybir.ActivationFunctionType.Identity,
                    bias=bias_sb[:, 0:1],
                )
                nc.scalar.dma_start(o_d[w, :, bh * 512 : (bh + 1) * 512], ot[:])

            # w = 0, 1: batch-halves kept separate so the first groups
            # only need the first half of x^T (prologue is HBM-bound)
            wt0 = load_slab(0)
            wt1 = load_slab(1)
            for c0, c1 in ((0, 9), (9, 18), (18, 26), (26, KCH)):
                nc.scalar.dma_start(
                    xt_sb[:, c0:c1, 512:BS], xt_src[:, c0:c1, 512:BS]
                )
            mm_group(wt0, 0, 0)
            mm_group(wt1, 1, 0)
            mm_group(wt0, 0, 1)
            mm_group(wt1, 1, 1)

            for w in range(2, N):
                wt = load_slab(w)
                mm_group(wt, w, 0)
                mm_group(wt, w, 1)

    nc.compile()
    return nc


def _get_module():
    if "nc" not in _CACHE:
        _CACHE["nc"] = _build_module()
    return _CACHE["nc"]


def kernel(x, adj, W, b, _trace=False):
    from concourse.bass_utils import run_bass_kernel_spmd

    x = np.ascontiguousarray(np.asarray(x, dtype=np.float32))
    adj = np.ascontiguousarray(np.asarray(adj, dtype=np.float32))
    W = np.ascontiguousarray(np.asarray(W, dtype=np.float32))
    b = np.ascontiguousarray(np.asarray(b, dtype=np.float32))

    nc = _get_module()

    # W pre-swizzled per m-half and cast to bf16:
    #   [w, p, h, kc, m'] = W[h, w, 2p+kc, mh*128+m']
    import ml_dtypes

    w_sw = []
    for mh in range(2):
        wh = W[:, :, :, mh * MH : (mh + 1) * MH]  # [h, w, n, m']
        wr = wh.reshape(N, N, FIN // 2, 2, MH)  # (h, w, p, kc, m')
        w_sw.append(
            np.ascontiguousarray(
                wr.transpose(1, 2, 0, 3, 4).astype(ml_dtypes.bfloat16)
            )
        )

    xt_by_bg = []
    for bg in range(NBG):
        xs = x[bg * BS : (bg + 1) * BS]  # [BS, N, FIN]
        # xt[(h*2+kc)*128 + p, b] = bf16(x[b, h, 2p+kc])
        xr = xs.reshape(BS, N, FIN // 2, 2)
        xt_by_bg.append(
            np.ascontiguousarray(
                xr.transpose(1, 3, 2, 0)
                .reshape(N * FIN, BS)
                .astype(ml_dtypes.bfloat16)
            )
        )

    in_maps = []
    for c in range(NC):
        bg, mh = divmod(c, 2)
        in_maps.append(
            {
                "xt": xt_by_bg[bg],
                "w_sw": w_sw[mh],
                "adj": adj,
                "b": b[mh * MH : (mh + 1) * MH].copy(),
            }
        )

    res = run_bass_kernel_spmd(nc, in_maps, list(range(NC)), trace=_trace)
    _CACHE["last_result"] = res

    out = np.empty((B, N, FOUT), dtype=np.float32)
    for c in range(NC):
        bg, mh = divmod(c, 2)
        ot = res.results[c]["out_t"]  # [17, 128, 1024] = (w, m', b)
        out[bg * BS : (bg + 1) * BS, :, mh * MH : (mh + 1) * MH] = ot.transpose(
            2, 0, 1
        )
    return out


# revision 30
# speedup vs baseline: 1.3400x; 1.0163x over previous
"""Trainium2 Bass kernel for NoSharingGraphConv.

out[b,w,m] = sum_{h,n} x[b,h,n] * adj[h,w] * W[h,w,n,m] + bias[m]
  B=4096, N=17 (graph nodes), FIN=FOUT=256.

Sharding (8 NeuronCores): 4 batch groups x 2 out-feature halves.
Core c handles batch rows [bg*1024, (bg+1)*1024) and out features
[mh*128, (mh+1)*128), bg = c>>1, mh = c&1. This halves the per-core W
stream (37.9MB) vs pure batch-parallel while keeping the PE work
perfectly balanced (1156 matmuls of [128x128]x[128x512] per core).

Device kernel (per core):
  - x^T shard resident in SBUF [128, 34, 1024] (host-transposed, n
    interleaved as n = 2p+kc so it matches the W slab layout).
  - W streamed one w-slab at a time; host pre-swizzles W into the exact
    slab layout [w, p, h, kc, m'] so each slab DMA is one fully
    contiguous 2.2MB read (17.4KB per partition line).
  - Slab scaled in-place by adj[:,w] on the DVE (per-h tensor_scalar,
    2x mode); adj is broadcast across partitions once via gpsimd.
  - Per (w, batch-half): 34 accumulating float32r matmuls into one PSUM
    bank; ACT evacuates with the per-partition bias add (fp32).
  - Device writes out_t [17, 128, 1024] (w, m', b); host permutes back.
"""

import sys

if "/opt/trn_rl_repo" not in sys.path:
    sys.path.insert(0, "/opt/trn_rl_repo")

import numpy as np

B, N, FIN, FOUT = 4096, 17, 256, 256
NC = 8
NBG = 4  # batch groups
BS = B // NBG  # 1024 batch rows per core
MH = FOUT // 2  # 128 out features per core
KCH = N * FIN // 128  # 34 contraction chunks of 128
NBH = BS // 512  # 2 batch halves (matmul free dim 512)

_CACHE = {}


def _build_module():
    import concourse.mybir as mybir
    import concourse.tile as tile
    from concourse import bacc

    f32 = mybir.dt.float32
    f32r = mybir.dt.float32r
    bf16 = mybir.dt.bfloat16

    nc = bacc.Bacc("TRN2", target_bir_lowering=False)

    # bf16 inputs: halves the dominant W DMA stream, halves the x^T
    # prologue load, and enables the PE fast-weight-load path (fp32
    # weight loads serialize at ~187ns/matmul). Accumulation stays fp32
    # in PSUM; walrus forbids mixing 16/32-bit matmul operands.
    xt_d = nc.dram_tensor("xt", [N * FIN, BS], bf16, kind="ExternalInput")
    # host-swizzled: w_sw[w, p, h, kc, m'] = bf16(W[h, w, 2p+kc, mh*128+m'])
    w_d = nc.dram_tensor("w_sw", [N, 128, N, 2, MH], bf16, kind="ExternalInput")
    # host-broadcast adj: adjb[p, w, h] = adj[h, w] for all 128 p
    adj_d = nc.dram_tensor("adjb", [128, N, N], f32, kind="ExternalInput")
    b_d = nc.dram_tensor("b", [MH], f32, kind="ExternalInput")
    o_d = nc.dram_tensor("out_t", [N, MH, BS], f32, kind="ExternalOutput")

    with tile.TileContext(nc) as tc:
        with (
            tc.tile_pool(name="const", bufs=1) as const,
            tc.tile_pool(name="wslab", bufs=3) as wpool,
            tc.tile_pool(name="obuf", bufs=4) as opool,
            tc.tile_pool(name="psum", bufs=6, space="PSUM") as psum,
        ):
            # PE warm-up: tiny junk matmuls during the prologue DMA
            # window release the HAM clock gate (1.2 -> 2.4 GHz) before
            # the real matmuls start. memset-fed, no DMA dependency.
            warm = const.tile([1, 512], bf16)
            nc.vector.memset(warm[:], 0.0)
            warm_ps = psum.tile([1, 512], f32, tag="ps")
            for _ in range(16):
                nc.tensor.matmul(
                    warm_ps[:], lhsT=warm[:, 0:1], rhs=warm[:], start=True, stop=True
                )

            # adj, already (w, h)-ordered and partition-broadcast by host
            adj_sb = const.tile([128, N, N], f32)  # [p][w][h]
            nc.sync.dma_start(adj_sb[:], adj_d[:])

            # bias half on partitions: bias_sb[p, 0] = b[mh*128 + p]
            bias_sb = const.tile([128, 1], f32)
            nc.sync.dma_start(bias_sb[:], b_d[:][:, None])

            # resident x^T, host-permuted: chunk c=(h,kc) row p holds
            # x[b, h, 2p+kc]. First batch-half loaded up front; the
            # second half is emitted after the first two w slabs so the
            # first matmul groups aren't starved. (ACT ring, so w-slab
            # loads on the SP ring run in parallel.)
            xt_sb = const.tile([128, KCH, BS], bf16)
            xt_src = xt_d[:].rearrange("(c p) b -> p c b", p=128)
            for c0, c1 in ((0, 9), (9, 18), (18, 26), (26, KCH)):
                nc.scalar.dma_start(xt_sb[:, c0:c1, 0:512], xt_src[:, c0:c1, 0:512])

            def load_slab(w):
                # one fully-contiguous 1.1MB slab read, then per-h
                # adj-scale on the DVE (bf16 tensor_scalar = 4x mode)
                wt = wpool.tile([128, N, 2, MH], bf16, tag="wslab")
                nc.sync.dma_start(
                    wt[:].rearrange("p h kc m -> p (h kc m)"),
                    w_d[w].rearrange("p h kc m -> p (h kc m)"),
                )
                for h in range(N):
                    nc.vector.tensor_scalar_mul(
                        wt[:, h].rearrange("p kc m -> p (kc m)"),
                        wt[:, h].rearrange("p kc m -> p (kc m)"),
                        adj_sb[:, w, h : h + 1],
                    )
                return wt

            def mm_group(wt, w, bh):
                ps = psum.tile([128, 512], mybir.dt.float32, tag="ps")
                for c in range(KCH):
                    h, kc = divmod(c, 2)
                    nc.tensor.matmul(
                        ps[:],
                        lhsT=wt[:, h, kc, :],
                        rhs=xt_sb[:, c, bh * 512 : (bh + 1) * 512],
                        start=(c == 0),
                        stop=(c == KCH - 1),
                    )
                evac(ps, w, bh)

            def evac(ps, w, bh):
                ot = opool.tile([128, 512], f32, tag="ot")
                nc.scalar.activation(
                    ot[:],
                    ps[:],
                    mybir.ActivationFunctionType.Identity,
                    bias=bias_sb[:, 0:1],
                )
                nc.scalar.dma_start(o_d[w, :, bh * 512 : (bh + 1) * 512], ot[:])

            # w = 0, 1: batch-halves kept separate so the first groups
            # only need the first half of x^T (prologue is HBM-bound)
            wt0 = load_slab(0)
            wt1 = load_slab(1)
            for c0, c1 in ((0, 9), (9, 18), (18, 26), (26, KCH)):
                nc.scalar.dma_start(
                    xt_sb[:, c0:c1, 512:BS], xt_src[:, c0:c1, 512:BS]
                )
            mm_group(wt0, 0, 0)
            mm_group(wt1, 1, 0)
            mm_group(wt0, 0, 1)
            mm_group(wt1, 1, 1)

            for w in range(2, N):
                wt = load_slab(w)
                mm_group(wt, w, 0)
                mm_group(wt, w, 1)

    nc.compile()
    return nc


def _get_module():
    if "nc" not in _CACHE:
        _CACHE["nc"] = _build_module()
    return _CACHE["nc"]


def kernel(x, adj, W, b, _trace=False):
    from concourse.bass_utils import run_bass_kernel_spmd

    x = np.ascontiguousarray(np.asarray(x, dtype=np.float32))
    adj = np.ascontiguousarray(np.asarray(adj, dtype=np.float32))
    W = np.ascontiguousarray(np.asarray(W, dtype=np.float32))
    b = np.ascontiguousarray(np.asarray(b, dtype=np.float32))

    nc = _get_module()

    # W pre-swizzled per m-half and cast to bf16:
    #   [w, p, h, kc, m'] = W[h, w, 2p+kc, mh*128+m']
    import ml_dtypes

    w_sw = []
    for mh in range(2):
        wh = W[:, :, :, mh * MH : (mh + 1) * MH]  # [h, w, n, m']
        wr = wh.reshape(N, N, FIN // 2, 2, MH)  # (h, w, p, kc, m')
        w_sw.append(
            np.ascontiguousarray(
                wr.transpose(1, 2, 0, 3, 4).astype(ml_dtypes.bfloat16)
            )
        )

    xt_by_bg = []
    for bg in range(NBG):
        xs = x[bg * BS : (bg + 1) * BS]  # [BS, N, FIN]
        # xt[(h*2+kc)*128 + p, b] = bf16(x[b, h, 2p+kc])
        xr = xs.reshape(BS, N, FIN // 2, 2)
        xt_by_bg.append(
            np.ascontiguousarray(
                xr.transpose(1, 3, 2, 0)
                .reshape(N * FIN, BS)
                .astype(ml_dtypes.bfloat16)
            )
        )

    # adjb[p, w, h] = adj[h, w], replicated across partitions
    adjb = np.ascontiguousarray(
        np.broadcast_to(adj.T[None, :, :], (128, N, N)).astype(np.float32)
    )

    in_maps = []
    for c in range(NC):
        bg, mh = divmod(c, 2)
        in_maps.append(
            {
                "xt": xt_by_bg[bg],
                "w_sw": w_sw[mh],
                "adjb": adjb,
                "b": b[mh * MH : (mh + 1) * MH].copy(),
            }
        )

    res = run_bass_kernel_spmd(nc, in_maps, list(range(NC)), trace=_trace)
    _CACHE["last_result"] = res

    out = np.empty((B, N, FOUT), dtype=np.float32)
    for c in range(NC):
        bg, mh = divmod(c, 2)
        ot = res.results[c]["out_t"]  # [17, 128, 1024] = (w, m', b)
        out[bg * BS : (bg + 1) * BS, :, mh * MH : (mh + 1) * MH] = ot.transpose(
            2, 0, 1
        )
    return out
